# revision 10
# baseline (speedup 1.0000x reference)
"""Causal self-attention with RoPE on 8 Trainium2 NeuronCores (v2, bf16).

Problem: B=2, S=2048, H=16 heads, D=128, HID=2048, fp32 reference.
  qkv = x @ w_qkv.T ; RoPE(q, k) ; causal softmax(q k^T / sqrt(D)) @ v ; out @ w_o.T

Sharding (hardcoded): core c handles batch b = c // 4 and head group
g = c % 4 (heads 4g..4g+4). Each core computes a partial (S, HID) output
contracted over its 512 hidden dims of the o-projection; the host sums the 4
partials per batch. Only the fp32 partials leave the device.

All matmul operands are bf16 (PSUM accumulation stays fp32): measured
end-to-end error vs the fp32 reference is ~3.4e-3 max-normalized, well under
the 2e-2 gate, and bf16 halves SBUF/DMA footprints (the whole 8 MB x panel
stays resident, so there is no mid-kernel reload stall), allows 1024-wide
moving operands in the projections, and gives DVE 2x modes for RoPE.

Phase A: q/k projected directly transposed (qT/kT [d=128, s]); weights are
host-prepped so every DMA is contiguous. x streams in 128-row chunks split
across two DMA queues so the first projection starts ~2us in. RoPE is applied
per (head, q/k) right after its projection (rotate-half via SBUF->SBUF DMA,
muls on DVE). v is projected in natural [s, 4*128] layout.

Phase B+C interleaved: scores^T per (sib si-block, head); softmax without max
subtraction (scores are O(5)); denominator via an all-ones [128,4] lhsT
matmul accumulated per head into one shared PSUM bank; 1/l as exp(-ln(l)) on
ACT, deferred by one head so it never head-blocks the exp stream in the ACT
FIFO; normalized attention output lands as outT[d, si] = exactly the lhsT
layout the o-projection needs. o-projection (phase C) work for si-block k is
interleaved into the attention rounds of block k+1 to fill PE gaps, and its
PSUM banks are DMA'd straight to DRAM (no staging copies).
"""

import os

import ml_dtypes
import numpy as np

import concourse.bacc as bacc
import concourse.tile as tile
from concourse import mybir
from concourse.bass_utils import run_bass_kernel_spmd

B, S, H, D = 2, 2048, 16, 128
HID = H * D
THETA = 10000.0
SCALE = 1.0 / float(np.sqrt(D))
NH = 4                 # heads per core
NC = 8                 # cores
NKC = HID // 128       # contraction chunks (128 wide)
SB = 512               # attention si-block / o-proj moving dim
NSB = S // SB          # si blocks
F32 = mybir.dt.float32
BF16 = mybir.dt.bfloat16
NPBF = ml_dtypes.bfloat16

MM_MODE = "bf16"
LAST_RESULT = None  # BassKernelResults of the most recent run (for test harness)


def _build_nc():
    nc = bacc.Bacc("TRN2", target_bir_lowering=False, debug=False, num_devices=NC)

    xT = nc.dram_tensor("xT", [HID, S], BF16, kind="ExternalInput")
    wqk = nc.dram_tensor("wqk", [2 * NH, 128, HID], BF16, kind="ExternalInput")
    wv = nc.dram_tensor("wv", [NKC, 128, NH * 128], BF16, kind="ExternalInput")
    woT = nc.dram_tensor("woT", [NH * 128, HID], BF16, kind="ExternalInput")
    cosT = nc.dram_tensor("cosT", [D, S], BF16, kind="ExternalInput")
    sinST = nc.dram_tensor("sinST", [D, S], BF16, kind="ExternalInput")
    maskadd = nc.dram_tensor("maskadd", [128, 128], F32, kind="ExternalInput")
    out = nc.dram_tensor("out", [S, HID], F32, kind="ExternalOutput")

    with tile.TileContext(nc) as tc:
        with tc.tile_pool(name="pconst", bufs=1) as pconst, \
             tc.tile_pool(name="px", bufs=1) as px, \
             tc.tile_pool(name="pqk", bufs=1) as pqk, \
             tc.tile_pool(name="pvn", bufs=1) as pvn, \
             tc.tile_pool(name="pwv", bufs=1) as pwvp, \
             tc.tile_pool(name="pwo", bufs=1) as pwo, \
             tc.tile_pool(name="pwq", bufs=4) as pwq, \
             tc.tile_pool(name="pst", bufs=3) as pst, \
             tc.tile_pool(name="ptrig", bufs=1) as ptrig:

            # ---- input DMAs (x split over two queues so chunk kc lands
            # ~0.75us*kc in; scalar queue carries the weights) ----
            xh = [px.tile([128, S], BF16, name=f"xh{kc}") for kc in range(NKC)]
            for kc in range(NKC):
                eng = nc.sync if kc % 2 == 0 else nc.gpsimd
                eng.dma_start(out=xh[kc], in_=xT[kc * 128:(kc + 1) * 128, :])

            wq_t = []
            for ot in range(2 * NH):
                wt = pwq.tile([128, HID], BF16, name="wqk")
                nc.scalar.dma_start(out=wt, in_=wqk[ot])
                wq_t.append(wt)
                if ot == 1:
                    cos_t = ptrig.tile([D, S], BF16, name="cosT")
                    sin_t = ptrig.tile([D, S], BF16, name="sinST")
                    tri_t = pconst.tile([128, 128], F32, name="tri")
                    nc.scalar.dma_start(out=cos_t, in_=cosT[:, :])
                    nc.scalar.dma_start(out=sin_t, in_=sinST[:, :])
                    nc.scalar.dma_start(out=tri_t, in_=maskadd[:, :])
            wv_t = []
            for kc in range(NKC):
                wvt = pwvp.tile([128, NH * 128], BF16, name=f"wv{kc}")
                nc.scalar.dma_start(out=wvt, in_=wv[kc])
                wv_t.append(wvt)
            wo_t = []
            for h in range(NH):
                wot = pwo.tile([128, HID], BF16, name=f"wo{h}")
                nc.scalar.dma_start(out=wot, in_=woT[h * 128:(h + 1) * 128, :])
                wo_t.append(wot)

            ones_f = pconst.tile([128, 4], F32, name="ones_f")
            nc.vector.memset(ones_f, 1.0)
            ones4 = pconst.tile([128, 4], BF16, name="ones4")
            nc.vector.tensor_copy(ones4, ones_f)

            qT = [pqk.tile([128, S], BF16, name=f"qT_{h}") for h in range(NH)]
            kT = [pqk.tile([128, S], BF16, name=f"kT_{h}") for h in range(NH)]
            vn = [pvn.tile([128, 4, NH * 128], BF16, name=f"vn_{g}")
                  for g in range(4)]
            # attention output aliases qT: q columns of an si-block are dead
            # once that block's scores are done.
            outT = qT

            # ---- Phase A1: q/k projection + RoPE ----
            with tc.tile_pool(name="ppa", bufs=8, space="PSUM") as ppa, \
                 tc.tile_pool(name="psh", bufs=2) as psh:
                for h in range(NH):
                    for kind, dst in ((0, qT[h]), (1, kT[h])):
                        wt = wq_t[kind * NH + h]
                        pss = [ppa.tile([128, SB], F32, name="qkps")
                               for _ in range(4)]
                        for kc in range(NKC):
                            for sb_i in range(4):
                                nc.tensor.matmul(
                                    pss[sb_i], wt[:, kc * 128:(kc + 1) * 128],
                                    xh[kc][:, sb_i * SB:(sb_i + 1) * SB],
                                    start=(kc == 0), stop=(kc == NKC - 1))
                        for sb_i in range(4):
                            sl = dst[:, sb_i * SB:(sb_i + 1) * SB]
                            if sb_i % 2 == 0:
                                nc.scalar.copy(out=sl, in_=pss[sb_i])
                            else:
                                nc.vector.tensor_copy(sl, pss[sb_i])
                        # RoPE in place (rotate-half partition swap via DMA)
                        sh_t = psh.tile([128, S], BF16, name="shuf")
                        nc.gpsimd.dma_start(out=sh_t[0:64, :], in_=dst[64:128, :])
                        nc.gpsimd.dma_start(out=sh_t[64:128, :], in_=dst[0:64, :])
                        nc.vector.tensor_mul(sh_t, sh_t, sin_t)
                        nc.vector.tensor_mul(dst, dst, cos_t)
                        nc.vector.tensor_add(dst, dst, sh_t)

            # ---- Phase A2: v projection, natural layout [s, 4 heads x d] ----
            with tc.tile_pool(name="pvp", bufs=8, space="PSUM") as pvp:
                for pas in range(4):
                    vps = [pvp.tile([128, NH * 128], F32, name="vps")
                           for _ in range(4)]
                    for kc in range(NKC):
                        for j in range(4):
                            st = pas * 4 + j
                            nc.tensor.matmul(
                                vps[j], xh[kc][:, st * 128:(st + 1) * 128],
                                wv_t[kc],
                                start=(kc == 0), stop=(kc == NKC - 1))
                    for j in range(4):
                        st = pas * 4 + j
                        dst = vn[st // 4][:, st % 4, :]
                        if j % 2 == 0:
                            nc.scalar.copy(out=dst, in_=vps[j])
                        else:
                            nc.vector.tensor_copy(dst, vps[j])

            # ---- Phase B (attention) with phase C (o-proj) interleaved ----
            c_queue = []       # (st, ob) o-proj units from the previous round
            pending = []       # deferred normalize: (h, si0, o_ps, l_row)
            c_idx = [0]        # alternates the staging-copy engine

            with tc.tile_pool(name="pexp", bufs=4) as pexp, \
                 tc.tile_pool(name="prr", bufs=2) as prr, \
                 tc.tile_pool(name="prb", bufs=2) as prb, \
                 tc.tile_pool(name="psc", bufs=2, space="PSUM") as psc, \
                 tc.tile_pool(name="plp", bufs=1, space="PSUM") as plp, \
                 tc.tile_pool(name="pop", bufs=2, space="PSUM") as pop, \
                 tc.tile_pool(name="pfp", bufs=1, space="PSUM") as pfp:

                def emit_norm():
                    h, si0, o_ps, l_row = pending.pop(0)
                    # 1/l as exp(-ln(l)): both live in one ACT table set
                    lnl = prr.tile([1, SB], F32, name="lnl")
                    nc.scalar.activation(out=lnl, in_=l_row,
                                         func=mybir.ActivationFunctionType.Ln)
                    rec = prr.tile([1, SB], F32, name="rec")
                    nc.scalar.activation(out=rec, in_=lnl,
                                         func=mybir.ActivationFunctionType.Exp,
                                         scale=-1.0)
                    rb = prb.tile([128, SB], F32, name="rb")
                    nc.gpsimd.partition_broadcast(rb, rec)
                    nc.vector.tensor_mul(outT[h][:, si0:si0 + SB], o_ps, rb)

                def emit_c_unit(pool):
                    st, ob = c_queue.pop(0)
                    fin = pool.tile([128, SB], F32, name="fin")
                    for hh in range(NH):
                        nc.tensor.matmul(
                            fin, outT[hh][:, st * 128:(st + 1) * 128],
                            wo_t[hh][:, ob * SB:(ob + 1) * SB],
                            start=(hh == 0), stop=(hh == NH - 1))
                    stg = pst.tile([128, SB], F32, name="stg")
                    if c_idx[0] % 2 == 0:
                        nc.vector.tensor_copy(stg, fin)
                    else:
                        nc.scalar.copy(out=stg, in_=fin)
                    c_idx[0] += 1
                    nc.sync.dma_start(
                        out=out[st * 128:(st + 1) * 128, ob * SB:(ob + 1) * SB],
                        in_=stg)

                for sib in range(NSB):
                    si0 = sib * SB
                    nch = 4 * (sib + 1)
                    l4 = plp.tile([4, SB], F32, name="l4")
                    for h in range(NH):
                        o_ps = pop.tile([128, SB], F32, name="ops")
                        for cp in range(nch // 2):
                            s_ps = psc.tile([128, 2, SB], F32, name="sps")
                            e_t = pexp.tile([128, 2, SB], BF16, name="exp")
                            los = []
                            for j in range(2):
                                cj = cp * 2 + j
                                dg = cj - (nch - 4)
                                lo = dg * 128 if dg > 0 else 0
                                los.append((cj, lo))
                                nc.tensor.matmul(
                                    s_ps[:, j, lo:],
                                    kT[h][:, cj * 128:(cj + 1) * 128],
                                    qT[h][:, si0 + lo:si0 + SB],
                                    start=True, stop=True)
                                if dg >= 0:
                                    nc.vector.tensor_add(
                                        s_ps[:, j, lo:lo + 128],
                                        s_ps[:, j, lo:lo + 128], tri_t)
                            # exp of the full tile: the sub-diagonal region is
                            # stale scores (or -1e30-masked), harmless and
                            # never read; one instruction keeps ACT dense.
                            nc.scalar.activation(
                                out=e_t, in_=s_ps,
                                func=mybir.ActivationFunctionType.Exp,
                                scale=SCALE)
                            # previous head's reciprocal goes behind this exp
                            # so it never head-blocks the ACT FIFO
                            if cp == 0 and pending:
                                emit_norm()
                            for j in range(2):
                                cj, lo = los[j]
                                nc.tensor.matmul(
                                    l4[:, lo:], ones4, e_t[:, j, lo:],
                                    start=(cj == 0), stop=(cj == nch - 1))
                                nc.tensor.matmul(
                                    o_ps[:, lo:],
                                    vn[cj // 4][:, cj % 4,
                                                h * 128:(h + 1) * 128],
                                    e_t[:, j, lo:],
                                    start=(cj == 0), stop=(cj == nch - 1))
                            if c_queue:
                                emit_c_unit(pfp)
                        pending.append((h, si0, o_ps, l4[0:1, :]))
                    # o-proj units for this now-complete si block run during
                    # the next round's attention
                    c_queue.extend([(sib * 4 + j, ob)
                                    for j in range(4) for ob in range(HID // SB)])
                if pending:
                    emit_norm()

            # ---- Phase C tail: last si-block's o-projection ----
            with tc.tile_pool(name="pft", bufs=4, space="PSUM") as pft:
                while c_queue:
                    emit_c_unit(pft)

    # Force exp and ln onto the single `natural_log_exp_and_others` ACT
    # table set: with the default map the table-load pass alternates between
    # the exp-only and ln-only sets (~2.7us per reload on ScalarE). Blank
    # the single-function sets (positions preserved, so set ids stay valid)
    # so both functions resolve to the combined set -> one load.
    import concourse.bacc as _bacc_mod
    import concourse.hw_specs as _hw_specs
    _orig_tables = _hw_specs.get_activation_tables

    def _patched_tables(arch):
        t = dict(_orig_tables(arch))
        for name in ("exp_and_others", "exp_and_friends", "natural_log"):
            if name in t:
                t[name] = set()
        return t

    _bacc_mod.get_activation_tables = _patched_tables
    try:
        nc.compile()
    finally:
        _bacc_mod.get_activation_tables = _orig_tables
    return nc


_NC_CACHE = None


def _get_nc():
    global _NC_CACHE
    if _NC_CACHE is None:
        _NC_CACHE = _build_nc()
    return _NC_CACHE


def _host_inputs(x, w_qkv, w_o):
    """Per-core input maps (sharding + layout prep + bf16 cast on host)."""
    inv_freq = 1.0 / (THETA ** (np.arange(0, D, 2, dtype=np.float64) / D))
    pos = np.arange(S, dtype=np.float64)
    freqs = pos[:, None] * inv_freq[None, :]          # (S, D/2)
    emb = np.concatenate([freqs, freqs], axis=-1)     # (S, D)
    cosT = np.ascontiguousarray(np.cos(emb).T).astype(NPBF)         # (D, S)
    sign = np.concatenate([-np.ones(D // 2), np.ones(D // 2)])
    sinST = np.ascontiguousarray((sign[None, :] * np.sin(emb)).T).astype(NPBF)
    # additive causal triangle for a diagonal 128x128 block of scores^T:
    # keep (add 0) when sj_local <= si_local, else -1e30
    p = np.arange(128)[:, None]
    f = np.arange(128)[None, :]
    maskadd = np.where(p <= f, 0.0, -1e30).astype(np.float32)       # (128, 128)

    xTb = [np.ascontiguousarray(x[b].T).astype(NPBF) for b in range(B)]
    in_maps = []
    for c in range(NC):
        b, g = c // 4, c % 4
        rows = slice(g * NH * D, (g + 1) * NH * D)
        wq = w_qkv[0 * HID:1 * HID][rows]             # (512, 2048)
        wk = w_qkv[1 * HID:2 * HID][rows]
        wvm = w_qkv[2 * HID:3 * HID][rows]
        # wqk[ot][p, kc*128+od] = w[ot*128+od, kc*128+p]
        wqk_arr = np.empty((2 * NH, 128, HID), dtype=NPBF)
        for kind, wm in ((0, wq), (1, wk)):
            for h in range(NH):
                wT = wm[h * 128:(h + 1) * 128].T      # (2048 hid, 128 od)
                wqk_arr[kind * NH + h] = (
                    wT.reshape(NKC, 128, 128).transpose(1, 0, 2)
                    .reshape(128, HID).astype(NPBF))
        # wv[kc][p, j] = w_v[j, kc*128+p]
        wv_arr = np.ascontiguousarray(
            wvm.T.reshape(NKC, 128, NH * 128)).astype(NPBF)
        woT = np.ascontiguousarray(w_o[:, rows].T).astype(NPBF)     # (512, HID)
        in_maps.append({
            "xT": xTb[b], "wqk": wqk_arr, "wv": wv_arr, "woT": woT,
            "cosT": cosT, "sinST": sinST, "maskadd": maskadd,
        })
    return in_maps


def kernel(x, w_qkv, w_o):
    global LAST_RESULT
    x = np.asarray(x, dtype=np.float32)
    w_qkv = np.asarray(w_qkv, dtype=np.float32)
    w_o = np.asarray(w_o, dtype=np.float32)

    nc = _get_nc()
    in_maps = _host_inputs(x, w_qkv, w_o)
    trace = bool(int(os.environ.get("BASS_KERNEL_TRACE", "0")))
    last_exc = None
    for _attempt in range(3):
        try:
            res = run_bass_kernel_spmd(
                nc, in_maps, core_ids=list(range(NC)),
                trace=trace, trace_cores=list(range(NC)) if trace else None)
            break
        except Exception as e:  # transient NRT device errors: retry
            last_exc = e
    else:
        raise last_exc
    LAST_RESULT = res

    out = np.empty((B, S, HID), dtype=np.float32)
    for b in range(B):
        acc = np.zeros((S, HID), dtype=np.float64)
        for g in range(4):
            acc += res.results[b * 4 + g]["out"]
        out[b] = acc.astype(np.float32)
    return out


# revision 19
# speedup vs baseline: 1.0254x; 1.0254x over previous
"""Causal self-attention with RoPE on 8 Trainium2 NeuronCores (v3).

Problem: B=2, S=2048, H=16 heads, D=128, HID=2048, fp32.
  qkv = x @ w_qkv.T ; RoPE(q, k) ; causal softmax(q k^T / sqrt(D)) @ v ; out @ w_o.T

Sharding (hardcoded): core c handles batch b = c // 4 and head group
g = c % 4 (heads 4g..4g+4). Each core computes a partial (S, HID) output
contracted over its 512 hidden dims of the o-projection; the host sums the 4
partials per batch.

All matmuls run in fp32r (TF32-class): at moving dims >=256 fp32r streams
1 column/cycle at the full 2.4 GHz clock, measurably faster than bf16
(bf16's higher PE power draw drops the clock to ~2.0 GHz: 259 ns vs 227 ns
per 512-col matmul, ~+46 us over this kernel's 733k columns).

Phase A (per s-half): q/k projected directly transposed (qT/kT [d, s]) with
host-prepped contiguous weight tiles; x streams in 128-row chunks split
across two DMA queues so the first matmul starts ~2 us in, and each chunk's
SBUF slot is recycled across halves chunk-by-chunk (no half-boundary stall).
RoPE is applied per (head, q/k) right after its half is projected. v is
projected in natural [s, 4*128] layout.

Phase B+C interleaved: scores^T per (si-block, head); softmax without max
subtraction (scores are O(5)); denominator rows for all 4 heads of a round
share one PSUM bank via an all-ones [128,4] lhsT; 1/l as exp(-ln(l)) on ACT,
deferred behind the next head's first exp so it never head-blocks the ACT
FIFO (except the round's last head, emitted at round end so the next round's
PE work never waits on it). o-projection units for si-block k run inside the
attention rounds of block k+1, gated off the first two chunk-pairs so the
normalize chain they depend on has completed.
"""

import os

import numpy as np

import concourse.bacc as bacc
import concourse.tile as tile
from concourse import mybir
from concourse.bass_utils import run_bass_kernel_spmd

B, S, H, D = 2, 2048, 16, 128
HID = H * D
THETA = 10000.0
SCALE = 1.0 / float(np.sqrt(D))
NH = 4                 # heads per core
NC = 8                 # cores
NKC = HID // 128       # contraction chunks (128 wide)
SB = 512               # attention si-block / moving dim
NSB = S // SB          # si blocks
SH = S // 2            # s-half
F32 = mybir.dt.float32

# "fp32r" -> TF32-class matmuls at full clock (rel err ~2e-4)
MM_MODE = os.environ.get("BASS_MM_MODE", "fp32r")
MMDT = mybir.dt.float32r if MM_MODE == "fp32r" else mybir.dt.float32

LAST_RESULT = None  # BassKernelResults of the most recent run (for test harness)


def _build_nc():
    nc = bacc.Bacc("TRN2", target_bir_lowering=False, debug=False, num_devices=NC)

    xT = nc.dram_tensor("xT", [HID, S], F32, kind="ExternalInput")
    wqk = nc.dram_tensor("wqk", [2 * NH, 128, HID], F32, kind="ExternalInput")
    wv = nc.dram_tensor("wv", [NKC, 128, NH * 128], F32, kind="ExternalInput")
    woT = nc.dram_tensor("woT", [NH * 128, HID], F32, kind="ExternalInput")
    cosT = nc.dram_tensor("cosT", [D, S], F32, kind="ExternalInput")
    sinST = nc.dram_tensor("sinST", [D, S], F32, kind="ExternalInput")
    maskadd = nc.dram_tensor("maskadd", [128, 128], F32, kind="ExternalInput")
    out = nc.dram_tensor("out", [S, HID], F32, kind="ExternalOutput")

    with tile.TileContext(nc) as tc:
        with tc.tile_pool(name="pconst", bufs=1) as pconst, \
             tc.tile_pool(name="pqk", bufs=1) as pqk, \
             tc.tile_pool(name="pvn", bufs=1) as pvn:

            ones_f = pconst.tile([128, 4], F32, name="ones_f")
            nc.vector.memset(ones_f, 1.0)
            ones4 = pconst.tile([128, 4], MMDT, name="ones4")
            nc.vector.tensor_copy(ones4, ones_f)
            tri_t = pconst.tile([128, 128], F32, name="tri")
            nc.scalar.dma_start(out=tri_t, in_=maskadd[:, :])

            qT = [pqk.tile([128, S], MMDT, name=f"qT_{h}") for h in range(NH)]
            kT = [pqk.tile([128, S], MMDT, name=f"kT_{h}") for h in range(NH)]
            vn = [pvn.tile([128, 4, NH * 128], MMDT, name=f"vn_{g}")
                  for g in range(4)]
            outT = qT  # attention output aliases qT per si-block

            # ---- Phase A: q/k/v projection + RoPE, per s-half ----
            with tc.tile_pool(name="px", bufs=1) as px, \
                 tc.tile_pool(name="pwq", bufs=3) as pwq, \
                 tc.tile_pool(name="pwv", bufs=4) as pwvp, \
                 tc.tile_pool(name="ptrig", bufs=1) as ptrig, \
                 tc.tile_pool(name="psh", bufs=1) as psh:
                for half in range(2):
                    s0 = half * SH
                    xh = []
                    for kc in range(NKC):
                        xt = px.tile([128, SH], MMDT, name=f"xh{kc}")
                        eng = nc.sync if kc % 2 == 0 else nc.gpsimd
                        eng.dma_start(
                            out=xt, in_=xT[kc * 128:(kc + 1) * 128,
                                           s0:s0 + SH].bitcast(MMDT))
                        xh.append(xt)
                    cos_t = ptrig.tile([D, SH], F32, name="cosT")
                    sin_t = ptrig.tile([D, SH], F32, name="sinST")
                    nc.scalar.dma_start(out=cos_t, in_=cosT[:, s0:s0 + SH])
                    nc.scalar.dma_start(out=sin_t, in_=sinST[:, s0:s0 + SH])

                    # q/k projection, transposed output [d, s], then RoPE
                    ppa_cm = tc.tile_pool(name="ppa", bufs=8, space="PSUM")
                    ppa = ppa_cm.__enter__()
                    for h in range(NH):
                        for kind, dst in ((0, qT[h]), (1, kT[h])):
                            ot = kind * NH + h
                            wt = pwq.tile([128, HID], MMDT, name="wqk")
                            nc.scalar.dma_start(out=wt,
                                                in_=wqk[ot].bitcast(MMDT))
                            ps0 = ppa.tile([128, SB], F32, name="qkps")
                            ps1 = ppa.tile([128, SB], F32, name="qkps")
                            for kc in range(NKC):
                                nc.tensor.matmul(
                                    ps0, wt[:, kc * 128:(kc + 1) * 128],
                                    xh[kc][:, 0:SB],
                                    start=(kc == 0), stop=(kc == NKC - 1))
                                nc.tensor.matmul(
                                    ps1, wt[:, kc * 128:(kc + 1) * 128],
                                    xh[kc][:, SB:SH],
                                    start=(kc == 0), stop=(kc == NKC - 1))
                            nc.scalar.copy(out=dst[:, s0:s0 + SB], in_=ps0)
                            nc.vector.tensor_copy(dst[:, s0 + SB:s0 + SH], ps1)
                            # RoPE in place (rotate-half partition swap by DMA)
                            sl = dst[:, s0:s0 + SH]
                            sh_t = psh.tile([128, SH], MMDT, name="shuf")
                            nc.gpsimd.dma_start(out=sh_t[0:64, :],
                                                in_=dst[64:128, s0:s0 + SH])
                            nc.gpsimd.dma_start(out=sh_t[64:128, :],
                                                in_=dst[0:64, s0:s0 + SH])
                            nc.vector.tensor_mul(sh_t, sh_t, sin_t)
                            nc.vector.tensor_mul(sl, sl, cos_t)
                            nc.vector.tensor_add(sl, sl, sh_t)

                    ppa_cm.__exit__(None, None, None)

                    # v projection, natural layout [s, 4 heads x d]
                    pvp_cm = tc.tile_pool(name="pvp", bufs=8, space="PSUM")
                    pvp = pvp_cm.__enter__()
                    wv_t = []
                    for kc in range(NKC):
                        wvt = pwvp.tile([128, NH * 128], MMDT, name="wv")
                        nc.scalar.dma_start(out=wvt, in_=wv[kc].bitcast(MMDT))
                        wv_t.append(wvt)
                    for pas in range(2):
                        vps = [pvp.tile([128, NH * 128], F32, name="vps")
                               for _ in range(4)]
                        for kc in range(NKC):
                            for j in range(4):
                                st = pas * 4 + j
                                nc.tensor.matmul(
                                    vps[j],
                                    xh[kc][:, st * 128:(st + 1) * 128],
                                    wv_t[kc],
                                    start=(kc == 0), stop=(kc == NKC - 1))
                        for j in range(4):
                            sg = half * 8 + pas * 4 + j   # global s-chunk
                            dst = vn[sg // 4][:, sg % 4, :]
                            if j % 2 == 0:
                                nc.scalar.copy(out=dst, in_=vps[j])
                            else:
                                nc.vector.tensor_copy(dst, vps[j])
                    pvp_cm.__exit__(None, None, None)

            # ---- Phase B (attention) with phase C (o-proj) interleaved ----
            pwo_cm = tc.tile_pool(name="pwo", bufs=1)
            pwo = pwo_cm.__enter__()
            pst_cm = tc.tile_pool(name="pst", bufs=3)
            pst = pst_cm.__enter__()
            wo_t = []
            for h in range(NH):
                wot = pwo.tile([128, HID], MMDT, name=f"wo{h}")
                nc.scalar.dma_start(out=wot,
                                    in_=woT[h * 128:(h + 1) * 128, :]
                                    .bitcast(MMDT))
                wo_t.append(wot)

            c_queue = []       # (st, ob) o-proj units from the previous round
            pending = []       # deferred normalize: (h, si0, o_ps, l_row)
            c_idx = [0]        # alternates the staging-copy engine

            with tc.tile_pool(name="pexp", bufs=4) as pexp, \
                 tc.tile_pool(name="prr", bufs=2) as prr, \
                 tc.tile_pool(name="prb", bufs=2) as prb, \
                 tc.tile_pool(name="psc", bufs=2, space="PSUM") as psc, \
                 tc.tile_pool(name="plp", bufs=1, space="PSUM") as plp, \
                 tc.tile_pool(name="pop", bufs=2, space="PSUM") as pop, \
                 tc.tile_pool(name="pfp", bufs=1, space="PSUM") as pfp:

                def emit_norm():
                    h, si0, o_ps, l_row = pending.pop(0)
                    # 1/l as exp(-ln(l)): both live in one ACT table set
                    lnl = prr.tile([1, SB], F32, name="lnl")
                    nc.scalar.activation(out=lnl, in_=l_row,
                                         func=mybir.ActivationFunctionType.Ln)
                    rec = prr.tile([1, SB], F32, name="rec")
                    nc.scalar.activation(out=rec, in_=lnl,
                                         func=mybir.ActivationFunctionType.Exp,
                                         scale=-1.0)
                    rb = prb.tile([128, SB], F32, name="rb")
                    nc.gpsimd.partition_broadcast(rb, rec)
                    nc.vector.tensor_mul(outT[h][:, si0:si0 + SB], o_ps, rb)

                def emit_c_unit(pool):
                    st, ob = c_queue.pop(0)
                    fin = pool.tile([128, SB], F32, name="fin")
                    for hh in range(NH):
                        nc.tensor.matmul(
                            fin, outT[hh][:, st * 128:(st + 1) * 128],
                            wo_t[hh][:, ob * SB:(ob + 1) * SB],
                            start=(hh == 0), stop=(hh == NH - 1))
                    stg = pst.tile([128, SB], F32, name="stg")
                    if c_idx[0] % 2 == 0:
                        nc.vector.tensor_copy(stg, fin)
                    else:
                        nc.scalar.copy(out=stg, in_=fin)
                    c_idx[0] += 1
                    nc.sync.dma_start(
                        out=out[st * 128:(st + 1) * 128, ob * SB:(ob + 1) * SB],
                        in_=stg)

                for sib in range(NSB):
                    si0 = sib * SB
                    nch = 4 * (sib + 1)
                    l4 = plp.tile([4, SB], F32, name="l4")
                    for h in range(NH):
                        o_ps = pop.tile([128, SB], F32, name="ops")
                        for cp in range(nch // 2):
                            s_ps = psc.tile([128, 2, SB], F32, name="sps")
                            e_t = pexp.tile([128, 2, SB], MMDT, name="exp")
                            los = []
                            for j in range(2):
                                cj = cp * 2 + j
                                dg = cj - (nch - 4)
                                lo = dg * 128 if dg > 0 else 0
                                los.append((cj, lo))
                                nc.tensor.matmul(
                                    s_ps[:, j, lo:],
                                    kT[h][:, cj * 128:(cj + 1) * 128],
                                    qT[h][:, si0 + lo:si0 + SB],
                                    start=True, stop=True)
                                if dg >= 0:
                                    nc.vector.tensor_add(
                                        s_ps[:, j, lo:lo + 128],
                                        s_ps[:, j, lo:lo + 128], tri_t)
                            # exp of the full tile: the sub-diagonal region is
                            # stale finite scores, harmless and never read;
                            # one instruction keeps ACT dense.
                            nc.scalar.activation(
                                out=e_t, in_=s_ps,
                                func=mybir.ActivationFunctionType.Exp,
                                scale=SCALE)
                            # previous head's reciprocal goes behind this exp
                            # so it never head-blocks the ACT FIFO
                            if cp == 0 and pending:
                                emit_norm()
                            for j in range(2):
                                cj, lo = los[j]
                                # all-ones [128,4] lhsT: every row of l4 gets
                                # this head's denominator; rows are overwritten
                                # per head (WAR-ordered after the ln read)
                                nc.tensor.matmul(
                                    l4[:, lo:], ones4, e_t[:, j, lo:],
                                    start=(cj == 0), stop=(cj == nch - 1))
                                nc.tensor.matmul(
                                    o_ps[:, lo:],
                                    vn[cj // 4][:, cj % 4,
                                                h * 128:(h + 1) * 128],
                                    e_t[:, j, lo:],
                                    start=(cj == 0), stop=(cj == nch - 1))
                            # o-proj units of the previous si block; skip the
                            # first pairs of a round so the normalize chain
                            # they read has completed
                            if c_queue and not (h == 0 and cp < 2):
                                emit_c_unit(pfp)
                        pending.append((h, si0, o_ps, l4[0:1, :]))
                    # last head's normalize at round end: the next round's l4
                    # reuses this bank, so don't leave its ln pending
                    emit_norm()
                    c_queue.extend([(sib * 4 + j, ob)
                                    for j in range(4) for ob in range(HID // SB)])
                while pending:
                    emit_norm()

            # ---- Phase C tail: last si-block's o-projection ----
            with tc.tile_pool(name="pft", bufs=4, space="PSUM") as pft:
                while c_queue:
                    emit_c_unit(pft)
            pst_cm.__exit__(None, None, None)
            pwo_cm.__exit__(None, None, None)

    # Force exp and ln onto the single `natural_log_exp_and_others` ACT
    # table set: with the default map the table-load pass alternates between
    # the exp-only and ln-only sets (~2.7us per reload on ScalarE). Blank
    # the single-function sets (positions preserved, so set ids stay valid)
    # so both functions resolve to the combined set -> one load.
    import concourse.bacc as _bacc_mod
    import concourse.hw_specs as _hw_specs
    _orig_tables = _hw_specs.get_activation_tables

    def _patched_tables(arch):
        t = dict(_orig_tables(arch))
        for name in ("exp_and_others", "exp_and_friends", "natural_log"):
            if name in t:
                t[name] = set()
        return t

    _bacc_mod.get_activation_tables = _patched_tables
    try:
        nc.compile()
    finally:
        _bacc_mod.get_activation_tables = _orig_tables
    return nc


_NC_CACHE = None


def _get_nc():
    global _NC_CACHE
    if _NC_CACHE is None:
        _NC_CACHE = _build_nc()
    return _NC_CACHE


def _host_inputs(x, w_qkv, w_o):
    """Per-core input maps (sharding + contiguous-DMA layout prep on host)."""
    inv_freq = 1.0 / (THETA ** (np.arange(0, D, 2, dtype=np.float64) / D))
    pos = np.arange(S, dtype=np.float64)
    freqs = pos[:, None] * inv_freq[None, :]          # (S, D/2)
    emb = np.concatenate([freqs, freqs], axis=-1)     # (S, D)
    cosT = np.ascontiguousarray(np.cos(emb).T.astype(np.float32))   # (D, S)
    sign = np.concatenate([-np.ones(D // 2), np.ones(D // 2)])
    sinST = np.ascontiguousarray((sign[None, :] * np.sin(emb)).T
                                 .astype(np.float32))               # (D, S)
    # additive causal triangle for a diagonal 128x128 block of scores^T:
    # keep (add 0) when sj_local <= si_local, else -1e30
    p = np.arange(128)[:, None]
    f = np.arange(128)[None, :]
    maskadd = np.where(p <= f, 0.0, -1e30).astype(np.float32)       # (128, 128)

    xTb = [np.ascontiguousarray(x[b].T) for b in range(B)]          # (HID, S)
    in_maps = []
    for c in range(NC):
        b, g = c // 4, c % 4
        rows = slice(g * NH * D, (g + 1) * NH * D)
        wq = w_qkv[0 * HID:1 * HID][rows]             # (512, 2048)
        wk = w_qkv[1 * HID:2 * HID][rows]
        wvm = w_qkv[2 * HID:3 * HID][rows]
        # wqk[ot][p, kc*128+od] = w[ot*128+od, kc*128+p]
        wqk_arr = np.empty((2 * NH, 128, HID), dtype=np.float32)
        for kind, wm in ((0, wq), (1, wk)):
            for h in range(NH):
                wT = wm[h * 128:(h + 1) * 128].T      # (2048 hid, 128 od)
                wqk_arr[kind * NH + h] = (
                    wT.reshape(NKC, 128, 128).transpose(1, 0, 2)
                    .reshape(128, HID))
        # wv[kc][p, j] = w_v[j, kc*128+p]
        wv_arr = np.ascontiguousarray(
            wvm.T.reshape(NKC, 128, NH * 128)).astype(np.float32)
        woT = np.ascontiguousarray(w_o[:, rows].T).astype(np.float32)
        in_maps.append({
            "xT": xTb[b], "wqk": wqk_arr, "wv": wv_arr, "woT": woT,
            "cosT": cosT, "sinST": sinST, "maskadd": maskadd,
        })
    return in_maps


def kernel(x, w_qkv, w_o):
    global LAST_RESULT
    x = np.asarray(x, dtype=np.float32)
    w_qkv = np.asarray(w_qkv, dtype=np.float32)
    w_o = np.asarray(w_o, dtype=np.float32)

    nc = _get_nc()
    in_maps = _host_inputs(x, w_qkv, w_o)
    trace = bool(int(os.environ.get("BASS_KERNEL_TRACE", "0")))
    last_exc = None
    for _attempt in range(3):
        try:
            res = run_bass_kernel_spmd(
                nc, in_maps, core_ids=list(range(NC)),
                trace=trace, trace_cores=list(range(NC)) if trace else None)
            break
        except Exception as e:  # transient NRT device errors: retry
            last_exc = e
    else:
        raise last_exc
    LAST_RESULT = res

    out = np.empty((B, S, HID), dtype=np.float32)
    for b in range(B):
        acc = np.zeros((S, HID), dtype=np.float64)
        for g in range(4):
            acc += res.results[b * 4 + g]["out"]
        out[b] = acc.astype(np.float32)
    return out
